# revision 26
# baseline (speedup 1.0000x reference)
"""Trainium2 Bass kernel for nn_CausalFeatureTransformer.

Only the label row (row 128) of the transformer output is returned by the
reference, so the per-node computation collapses to:

  zn    = LN(Z[n])                                  (over 128 feats)
  s     = zn / sqrt(zn^2 * var_f + eps)             (per feature)
  score = G[h,j] * s[n,j] + D[h,j]                  (label-query attention)
  p     = softmax_j(score)   (max-free: |score| <~ 8)
  num_h = sum_j p*s*Cv_h[j]  den_h = sum_j e        (+ label-token consts)
  x     = (num/den) @ wo + c0
  y     = x + gelu(LN(x) @ w1' + b1') @ w2 + b2

where G, D, Cv, c0, w1', b1', ... are O(params) constants folded on the host
(weight preprocessing; independent of the batch data Z).

Sharding: pure data-parallel over nodes N: each of the 8 cores processes a
512-node shard of Z; the small folded params are replicated. The device
output is (64, 512) node-major-last; the host transposes on gather.

Device-side notes:
 - rsqrt is computed as Exp(-0.5*Ln(x)) so every ACT func used before the
   final Gelu lives in one activation-table set (no table reload thrash).
 - big reciprocals use the custom-DVE reciprocal_approx_fast (~18 bits).
 - per-head attention sums land at PSUM partition base 32*p (HW constraint:
   matmul out base must be 0/32/64); junk rows are zero-folded via wo.
 - matmul operands are bf16 (PSUM accumulation stays fp32).
"""

import math

import numpy as np

D_FEAT, D_EMB, H, DK = 128, 64, 4, 16
SEQ = D_FEAT + 1
N = 4096
N_CORES = 8
NS = N // N_CORES  # 512 nodes per core
EPS = 1e-5

_CACHE = {}


def _ln64(x, eps=EPS):
    m = x.mean(-1, keepdims=True)
    v = ((x - m) ** 2).mean(-1, keepdims=True)
    return (x - m) / np.sqrt(v + eps)


def _host_consts(A_full, feat_emb, label_token, wq, bq, wk, bk, wv, bv, wo, bo,
                 w1, b1, w2, b2, alpha, g1, be1, g2, be2):
    """Fold all O(params) quantities on the host (float64 for stability)."""
    import ml_dtypes
    d = np.float64
    fe = feat_emb.astype(d)
    mu = fe.mean(1, keepdims=True)
    vf = ((fe - mu) ** 2).mean(1)                    # (128,)
    cf = (fe - mu) * g1.astype(d)                    # (128,64)

    t = _ln64(label_token.astype(d)[0, 0]) * g1.astype(d) + be1.astype(d)
    qlab = t @ wq.astype(d) + bq.astype(d)
    klab = t @ wk.astype(d) + bk.astype(d)
    vlab = t @ wv.astype(d) + bv.astype(d)

    Ck = cf @ wk.astype(d)                           # (128,64)
    Cv = cf @ wv.astype(d)                           # (128,64)
    bk_p = be1.astype(d) @ wk.astype(d) + bk.astype(d)
    bv_p = be1.astype(d) @ wv.astype(d) + bv.astype(d)

    al = float(alpha)
    rdk = 1.0 / math.sqrt(DK)
    G = np.zeros((H, D_FEAT), d)
    Dm = np.zeros((H, D_FEAT), d)
    slab = np.zeros(H, d)
    for h in range(H):
        blk = slice(h * DK, (h + 1) * DK)
        G[h] = Ck[:, blk] @ qlab[blk] * rdk
        Dm[h] = qlab[blk] @ bk_p[blk] * rdk + al * A_full[:D_FEAT, D_FEAT].astype(d)
        slab[h] = qlab[blk] @ klab[blk] * rdk + al * A_full[D_FEAT, D_FEAT]
    elab = np.exp(slab)                              # (4,)

    c0 = label_token.astype(d)[0, 0] + bv_p @ wo.astype(d) + bo.astype(d)
    w1p = w1.astype(d) * g2.astype(d)[:, None]       # diag(g2) @ w1
    b1p = be2.astype(d) @ w1.astype(d) + b1.astype(d)

    # Head h lives in pair pr=h//2 at PSUM partition base 32*(h%2).
    nbias = elab[:, None] * (vlab - bv_p).reshape(H, DK)     # (4,16)
    numbias = np.zeros((2, 64))
    denbias = np.zeros((2, 64))
    wo_exp = np.zeros((128, D_EMB))                  # [pairA(64) ; pairB(64)]
    for h in range(H):
        pr, p = divmod(h, 2)
        numbias[pr, 32 * p:32 * p + DK] = nbias[h]
        denbias[pr, 32 * p:32 * p + 32] = elab[h]
        wo_exp[64 * pr + 32 * p:64 * pr + 32 * p + DK] = \
            wo.astype(d)[h * DK:(h + 1) * DK]

    f32 = np.float32
    bf16 = ml_dtypes.bfloat16

    # fp32 blob (128, 204): sqrtvf | G | D | b1p | (c0+b2) | ident | c0-bcast
    blob_f = np.zeros((128, 204), f32)
    blob_f[:, 0] = np.sqrt(vf)
    blob_f[:, 1:5] = G.T
    blob_f[:, 5:9] = Dm.T
    blob_f[:, 9] = b1p
    blob_f[:64, 10] = c0 + b2
    blob_f[:, 12:140] = np.eye(128, dtype=f32)
    blob_f[:, 140:204] = c0[None, :]

    # bf16 blob (128, 448): cv | w2 | wo_exp | w1p (rows 0:64) | ident_bf16
    blob_b = np.zeros((128, 448), bf16)
    blob_b[:, 0:64] = Cv.astype(bf16)
    blob_b[:, 64:128] = w2.astype(bf16)
    blob_b[:, 128:192] = wo_exp.astype(bf16)
    blob_b[:64, 192:320] = w1p.astype(bf16)
    blob_b[:, 320:448] = np.eye(128, dtype=bf16)

    # bf16 row blob (1, 256): numbias_a/b | denbias_a/b
    blob_r = np.zeros((1, 256), bf16)
    blob_r[0, 0:64] = numbias[0].astype(bf16)
    blob_r[0, 64:128] = numbias[1].astype(bf16)
    blob_r[0, 128:192] = denbias[0].astype(bf16)
    blob_r[0, 192:256] = denbias[1].astype(bf16)

    return {"blob_f": blob_f, "blob_b": blob_b, "blob_r": blob_r}


def _build_bass():
    import concourse.bacc as bacc
    import concourse.mybir as mybir
    import concourse.tile as tile
    from concourse.masks import make_identity

    f32 = mybir.dt.float32
    bf16 = mybir.dt.bfloat16
    AF = mybir.ActivationFunctionType
    OP = mybir.AluOpType

    # Restrict Ln/Exp to the one table set containing both, so the
    # act-table-load pass cannot ping-pong between sets (each reload ~1.3us).
    # Set positions/ids are unchanged; we only narrow the choices.
    import concourse.hw_specs as hw_specs
    _orig_gat = hw_specs.get_activation_tables

    def _gat(arch):
        t = {k: set(v) for k, v in _orig_gat(arch).items()}
        for name, funcs in t.items():
            if name != "natural_log_exp_and_others":
                funcs.discard(AF.Exp)
                funcs.discard(AF.Ln)
        return t

    bacc.get_activation_tables = _gat

    nc = bacc.Bacc("TRN2", target_bir_lowering=False, debug=False,
                   num_devices=N_CORES)

    zs = nc.dram_tensor("zs", (NS, D_FEAT), f32, kind="ExternalInput")
    blob_f_d = nc.dram_tensor("blob_f", (128, 204), f32, kind="ExternalInput")
    blob_b_d = nc.dram_tensor("blob_b", (128, 448), bf16, kind="ExternalInput")
    blob_r_d = nc.dram_tensor("blob_r", (1, 256), bf16, kind="ExternalInput")
    yt = nc.dram_tensor("yt", (D_EMB, NS), f32, kind="ExternalOutput")

    with tile.TileContext(nc) as tc:
        with (
            tc.tile_pool(name="cp", bufs=1) as cp,
            tc.tile_pool(name="wk", bufs=1) as wkp,
            tc.tile_pool(name="sm", bufs=2) as sm,
            tc.tile_pool(name="hd", bufs=3) as hd,
            tc.tile_pool(name="ps", bufs=1, space="PSUM") as ps,
        ):
            # Z shard first, split per chunk, on the scalar queue (its
            # preamble ends earliest) so compute starts ASAP.
            za = wkp.tile([128, 4, D_FEAT], f32, tag="za")
            zre = zs.rearrange("(t p) f -> p t f", p=128)
            nc.sync.dma_start(out=za[:, 0:2, :], in_=zre[:, 0:2, :])
            nc.sync.dma_start(out=za[:, 2:4, :], in_=zre[:, 2:4, :])
            bf = cp.tile([128, 204], f32, tag="bf", name="bf")
            nc.sync.dma_start(out=bf, in_=blob_f_d[:])
            bb = cp.tile([128, 448], bf16, tag="bb", name="bb")
            nc.sync.dma_start(out=bb, in_=blob_b_d[:])
            br = cp.tile([1, 256], bf16, tag="br", name="br")
            nc.sync.dma_start(out=br, in_=blob_r_d[:])

            sqrtvf = bf[:, 0:1]
            gcol = bf[:, 1:5]
            dcol = bf[:, 5:9]
            b1p = bf[:, 9:10]
            c0b2 = bf[:64, 10:11]
            ident = bf[:, 12:140]
            c0b = bf[:, 140:204]
            cv = bb[:, 0:64]
            w2m = bb[:, 64:128]
            wo_m = bb[:, 128:192]
            w1p = bb[:64, 192:320]
            identb = bb[:, 320:448]

            ones_row = cp.tile([1, NS], bf16, tag="ones_row")
            nc.vector.memset(ones_row, 1.0)
            ones32 = cp.tile([128, 32], bf16, tag="ones32")
            nc.vector.memset(ones32, 1.0)
            eps_t = cp.tile([128, 1], f32, tag="eps_t")
            nc.vector.memset(eps_t, EPS)

            # ==== two node-chunks of 256, phase-interleaved so each engine
            # ping-pongs between chunks (software pipeline).
            NCH, CH, TPC = 2, NS // 2, 2
            C = range(NCH)
            st = {}

            # -- phase 1: LN(Z) stats
            for c in C:
                mvall = sm.tile([128, TPC, 2], f32, tag=f"mvall{c}", bufs=1,
                                name="mvall")
                st[c, "mv"] = mvall
                for t in range(TPC):
                    st6 = sm.tile([128, 6], f32, tag="st6", name="st6")
                    nc.vector.bn_stats(out=st6, in_=za[:, TPC * c + t, :])
                    nc.vector.bn_aggr(out=mvall[:, t, :], in_=st6)
            for c in C:
                lnv = sm.tile([128, TPC], f32, tag="lnv", name="lnv")
                nc.scalar.activation(out=lnv, in_=st[c, "mv"][:, :, 1],
                                     func=AF.Ln, bias=eps_t)
                rstd = sm.tile([128, TPC], f32, tag=f"rstd{c}", bufs=1,
                               name="rstd")
                nc.scalar.activation(out=rstd, in_=lnv, func=AF.Exp, scale=-0.5)
                st[c, "rstd"] = rstd

            # -- zn + transpose to layout B (zn on ACT: za*rstd - m*rstd)
            for c in C:
                mr = sm.tile([128, TPC], f32, tag="mr", name="mr")
                nc.vector.tensor_mul(out=mr, in0=st[c, "mv"][:, :, 0],
                                     in1=st[c, "rstd"])
                negmr = sm.tile([128, TPC], f32, tag=f"negmr{c}", bufs=1,
                                name="negmr")
                nc.vector.tensor_scalar_mul(out=negmr, in0=mr, scalar1=-1.0)
                st[c, "negmr"] = negmr
            for c in C:
                znT_ps = ps.tile([128, CH], f32, tag=f"pA{c}", name="znT_ps")
                st[c, "znT"] = znT_ps
                for t in range(TPC):
                    zn = sm.tile([128, D_FEAT], f32, tag="zn", name="zn")
                    nc.scalar.activation(
                        out=zn, in_=za[:, TPC * c + t, :], func=AF.Identity,
                        scale=st[c, "rstd"][:, t:t + 1],
                        bias=st[c, "negmr"][:, t:t + 1])
                    nc.tensor.transpose(znT_ps[:, t * 128:(t + 1) * 128], zn,
                                        ident)

            # -- s = zn * rsqrt(zn^2*vf + eps)
            for c in C:
                zsq = wkp.tile([128, CH], f32, tag=f"zsq{c}", name="zsq")
                nc.scalar.activation(out=zsq, in_=st[c, "znT"], func=AF.Square,
                                     scale=sqrtvf, bias=0.0)
                lns = wkp.tile([128, CH], f32, tag=f"lns{c}", name="lns")
                nc.scalar.activation(out=lns, in_=zsq, func=AF.Ln, bias=eps_t)
                rr = wkp.tile([128, CH], f32, tag=f"rr{c}", name="rr")
                nc.scalar.activation(out=rr, in_=lns, func=AF.Exp, scale=-0.5)
                sT = wkp.tile([128, CH], bf16, tag=f"sT{c}", name="sT")
                nc.vector.tensor_mul(out=sT, in0=st[c, "znT"], in1=rr)
                st[c, "sT"] = sT

            # -- attention: head h -> pair pr=h//2 at base 32*(h%2)
            for c in C:
                st[c, "num"] = [
                    ps.tile([D_EMB, CH], f32, tag=f"pB{c}", name="num_psa"),
                    ps.tile([D_EMB, CH], f32, tag=f"pC{c}", name="num_psb")]
                st[c, "den"] = [
                    ps.tile([D_EMB, CH], f32, tag=f"pA{c}", name="den_psa"),
                    ps.tile([D_EMB, CH], f32, tag=f"pD{c}", name="den_psb")]
                for pr in (0, 1):
                    nc.tensor.matmul(st[c, "num"][pr][:, :],
                                     br[:, 64 * pr:64 * pr + 64],
                                     ones_row[:, :CH], start=True, stop=False)
                    nc.tensor.matmul(st[c, "den"][pr][:, :],
                                     br[:, 128 + 64 * pr:192 + 64 * pr],
                                     ones_row[:, :CH], start=True, stop=False)
            for h in range(4):
                pr, p = divmod(h, 2)
                for c in C:
                    eh = hd.tile([128, CH], bf16, tag="eh", name="eh")
                    nc.scalar.activation(out=eh, in_=st[c, "sT"], func=AF.Exp,
                                         scale=gcol[:, h:h + 1],
                                         bias=dcol[:, h:h + 1])
                    esh = hd.tile([128, CH], bf16, tag="esh", name="esh")
                    nc.vector.tensor_mul(out=esh, in0=eh, in1=st[c, "sT"])
                    nc.tensor.matmul(st[c, "num"][pr][32 * p:32 * p + DK, :],
                                     cv[:, h * DK:(h + 1) * DK], esh,
                                     start=False, stop=(p == 1))
                    nc.tensor.matmul(st[c, "den"][pr][32 * p:32 * p + 32, :],
                                     ones32, eh, start=False, stop=(p == 1))

            # -- oe = num/den (stacked pairs), x in both layouts
            for c in C:
                oe = wkp.tile([128, CH], bf16, tag=f"oe{c}", name="oe")
                st[c, "oe"] = oe
                for pr in (0, 1):
                    rcp = wkp.tile([D_EMB, CH], f32, tag=f"rcp{pr}", name="rcp")
                    nc.vector.reciprocal_approx_fast(out=rcp,
                                                     in_=st[c, "den"][pr])
                    nc.vector.tensor_mul(out=oe[64 * pr:64 * pr + 64, :],
                                         in0=st[c, "num"][pr], in1=rcp)
            for c in C:
                x_ps = ps.tile([D_EMB, CH], f32, tag=f"pD{c}", name="x_ps")
                st[c, "x"] = x_ps
                nc.tensor.matmul(x_ps, wo_m, st[c, "oe"], start=True,
                                 stop=False)
                xa_ps = ps.tile([128, TPC, D_EMB], f32, tag=f"pA{c}",
                                name="xa_ps")
                st[c, "xap"] = xa_ps
                for t in range(TPC):
                    nc.tensor.matmul(xa_ps[:, t, :],
                                     st[c, "oe"][:, t * 128:(t + 1) * 128],
                                     wo_m, start=True, stop=True)
            for c in C:
                xa = wkp.tile([128, TPC, D_EMB], f32, tag=f"xa{c}", name="xa")
                st[c, "xa"] = xa
                nc.vector.tensor_add(
                    out=xa, in0=st[c, "xap"],
                    in1=c0b.unsqueeze(1).to_broadcast((128, TPC, D_EMB)))

            # -- FFN layernorm stats
            for c in C:
                mvb = sm.tile([128, TPC, 2], f32, tag=f"mvb{c}", bufs=1,
                              name="mvb")
                st[c, "mvb"] = mvb
                for t in range(TPC):
                    st6b = sm.tile([128, 6], f32, tag="st6b", name="st6b")
                    nc.vector.bn_stats(out=st6b, in_=st[c, "xa"][:, t, :])
                    nc.vector.bn_aggr(out=mvb[:, t, :], in_=st6b)
            for c in C:
                lnvb = sm.tile([128, TPC], f32, tag="lnvb", name="lnvb")
                nc.scalar.activation(out=lnvb, in_=st[c, "mvb"][:, :, 1],
                                     func=AF.Ln, bias=eps_t)
                rstdb = sm.tile([128, TPC], f32, tag=f"rstdb{c}", bufs=1,
                                name="rstdb")
                nc.scalar.activation(out=rstdb, in_=lnvb, func=AF.Exp,
                                     scale=-0.5)
                st[c, "rstdb"] = rstdb

            # -- u-hat + transpose back to layout B
            for c in C:
                uT_ps = ps.tile([D_EMB, CH], bf16, tag=f"pB{c}", name="uT_ps")
                st[c, "uTp"] = uT_ps
                for t in range(TPC):
                    uh = sm.tile([128, D_EMB], bf16, tag="uh", name="uh")
                    nc.vector.tensor_scalar(
                        out=uh, in0=st[c, "xa"][:, t, :],
                        scalar1=st[c, "mvb"][:, t, 0:1],
                        scalar2=st[c, "rstdb"][:, t:t + 1],
                        op0=OP.subtract, op1=OP.mult)
                    nc.tensor.transpose(uT_ps[:, t * 128:(t + 1) * 128], uh,
                                        identb)
            for c in C:
                uT = wkp.tile([D_EMB, CH], bf16, tag=f"uT{c}", name="uT")
                st[c, "uT"] = uT
                nc.vector.tensor_copy(out=uT, in_=st[c, "uTp"])

            # -- FFN matmuls; w2 accumulates into x_ps; y = x_ps + (c0+b2)
            for c in C:
                h_ps = ps.tile([2 * D_EMB, CH], f32, tag=f"pC{c}", name="h_ps")
                st[c, "h"] = h_ps
                nc.tensor.matmul(h_ps, w1p, st[c, "uT"], start=True, stop=True)
            for c in C:
                hh = wkp.tile([2 * D_EMB, CH], bf16, tag=f"hh{c}", name="hh")
                nc.scalar.activation(out=hh, in_=st[c, "h"], func=AF.Gelu,
                                     bias=b1p)
                nc.tensor.matmul(st[c, "x"], w2m, hh, start=False, stop=True)
            for c in C:
                y_sb = wkp.tile([D_EMB, CH], f32, tag=f"y_sb{c}", name="y_sb")
                nc.vector.tensor_scalar_add(out=y_sb, in0=st[c, "x"],
                                            scalar1=c0b2)
                nc.sync.dma_start(out=yt[:, c * CH:(c + 1) * CH], in_=y_sb)

    nc.compile()
    return nc


def _get_nc():
    if "nc" not in _CACHE:
        _CACHE["nc"] = _build_bass()
    return _CACHE["nc"]


def kernel(Z, A_full, feat_emb, label_token, wq, bq, wk, bk, wv, bv, wo, bo,
           w1, b1, w2, b2, alpha, g1, be1, g2, be2, _trace=False,
           _trace_kwargs=None):
    from concourse.bass_utils import run_bass_kernel_spmd

    Z = np.ascontiguousarray(np.asarray(Z, dtype=np.float32))
    consts = _host_consts(
        np.asarray(A_full), np.asarray(feat_emb), np.asarray(label_token),
        np.asarray(wq), np.asarray(bq), np.asarray(wk), np.asarray(bk),
        np.asarray(wv), np.asarray(bv), np.asarray(wo), np.asarray(bo),
        np.asarray(w1), np.asarray(b1), np.asarray(w2), np.asarray(b2),
        np.asarray(alpha), np.asarray(g1), np.asarray(be1), np.asarray(g2),
        np.asarray(be2))
    consts = {k: np.ascontiguousarray(v) for k, v in consts.items()}

    nc = _get_nc()
    in_maps = []
    for c in range(N_CORES):
        m = dict(consts)
        m["zs"] = np.ascontiguousarray(Z[c * NS:(c + 1) * NS])
        in_maps.append(m)

    kw = {}
    if _trace:
        kw["trace"] = True
        if _trace_kwargs:
            kw.update(_trace_kwargs)
    res = run_bass_kernel_spmd(nc, in_maps, core_ids=list(range(N_CORES)), **kw)

    out = np.empty((N, D_EMB), np.float32)
    for c in range(N_CORES):
        out[c * NS:(c + 1) * NS] = res.results[c]["yt"].T
    if _trace:
        return out, res
    return out


# revision 27
# speedup vs baseline: 1.0357x; 1.0357x over previous
"""Trainium2 Bass kernel for nn_CausalFeatureTransformer.

Only the label row (row 128) of the transformer output is returned by the
reference, so the per-node computation collapses to:

  zn    = LN(Z[n])                                  (over 128 feats)
  s     = zn / sqrt(zn^2 * var_f + eps)             (per feature)
  score = G[h,j] * s[n,j] + D[h,j]                  (label-query attention)
  p     = softmax_j(score)   (max-free: |score| <~ 8)
  num_h = sum_j p*s*Cv_h[j]  den_h = sum_j e        (+ label-token consts)
  x     = (num/den) @ wo + c0
  y     = x + gelu(LN(x) @ w1' + b1') @ w2 + b2

where G, D, Cv, c0, w1', b1', ... are O(params) constants folded on the host
(weight preprocessing; independent of the batch data Z).

Sharding: pure data-parallel over nodes N: each of the 8 cores processes a
512-node shard of Z; the small folded params are replicated. The device
output is (64, 512) node-major-last; the host transposes on gather.

Device-side notes:
 - rsqrt is computed as Exp(-0.5*Ln(x)) so every ACT func used before the
   final Gelu lives in one activation-table set (no table reload thrash).
 - big reciprocals use the custom-DVE reciprocal_approx_fast (~18 bits).
 - per-head attention sums land at PSUM partition base 32*p (HW constraint:
   matmul out base must be 0/32/64); junk rows are zero-folded via wo.
 - matmul operands are bf16 (PSUM accumulation stays fp32).
"""

import math

import numpy as np

D_FEAT, D_EMB, H, DK = 128, 64, 4, 16
SEQ = D_FEAT + 1
N = 4096
N_CORES = 8
NS = N // N_CORES  # 512 nodes per core
EPS = 1e-5

_CACHE = {}


def _ln64(x, eps=EPS):
    m = x.mean(-1, keepdims=True)
    v = ((x - m) ** 2).mean(-1, keepdims=True)
    return (x - m) / np.sqrt(v + eps)


def _host_consts(A_full, feat_emb, label_token, wq, bq, wk, bk, wv, bv, wo, bo,
                 w1, b1, w2, b2, alpha, g1, be1, g2, be2):
    """Fold all O(params) quantities on the host (float64 for stability)."""
    import ml_dtypes
    d = np.float64
    fe = feat_emb.astype(d)
    mu = fe.mean(1, keepdims=True)
    vf = ((fe - mu) ** 2).mean(1)                    # (128,)
    cf = (fe - mu) * g1.astype(d)                    # (128,64)

    t = _ln64(label_token.astype(d)[0, 0]) * g1.astype(d) + be1.astype(d)
    qlab = t @ wq.astype(d) + bq.astype(d)
    klab = t @ wk.astype(d) + bk.astype(d)
    vlab = t @ wv.astype(d) + bv.astype(d)

    Ck = cf @ wk.astype(d)                           # (128,64)
    Cv = cf @ wv.astype(d)                           # (128,64)
    bk_p = be1.astype(d) @ wk.astype(d) + bk.astype(d)
    bv_p = be1.astype(d) @ wv.astype(d) + bv.astype(d)

    al = float(alpha)
    rdk = 1.0 / math.sqrt(DK)
    G = np.zeros((H, D_FEAT), d)
    Dm = np.zeros((H, D_FEAT), d)
    slab = np.zeros(H, d)
    for h in range(H):
        blk = slice(h * DK, (h + 1) * DK)
        G[h] = Ck[:, blk] @ qlab[blk] * rdk
        Dm[h] = qlab[blk] @ bk_p[blk] * rdk + al * A_full[:D_FEAT, D_FEAT].astype(d)
        slab[h] = qlab[blk] @ klab[blk] * rdk + al * A_full[D_FEAT, D_FEAT]
    elab = np.exp(slab)                              # (4,)

    c0 = label_token.astype(d)[0, 0] + bv_p @ wo.astype(d) + bo.astype(d)
    w1p = w1.astype(d) * g2.astype(d)[:, None]       # diag(g2) @ w1
    b1p = be2.astype(d) @ w1.astype(d) + b1.astype(d)

    # Head h lives in pair pr=h//2 at PSUM partition base 32*(h%2).
    nbias = elab[:, None] * (vlab - bv_p).reshape(H, DK)     # (4,16)
    numbias = np.zeros((2, 64))
    denbias = np.zeros((2, 64))
    wo_exp = np.zeros((128, D_EMB))                  # [pairA(64) ; pairB(64)]
    for h in range(H):
        pr, p = divmod(h, 2)
        numbias[pr, 32 * p:32 * p + DK] = nbias[h]
        denbias[pr, 32 * p:32 * p + 32] = elab[h]
        wo_exp[64 * pr + 32 * p:64 * pr + 32 * p + DK] = \
            wo.astype(d)[h * DK:(h + 1) * DK]

    f32 = np.float32
    bf16 = ml_dtypes.bfloat16

    # fp32 blob (128, 204): sqrtvf | G | D | b1p | (c0+b2) | ident | c0-bcast
    blob_f = np.zeros((128, 204), f32)
    blob_f[:, 0] = np.sqrt(vf)
    blob_f[:, 1:5] = G.T
    blob_f[:, 5:9] = Dm.T
    blob_f[:, 9] = b1p
    blob_f[:64, 10] = c0 + b2
    blob_f[:, 12:140] = np.eye(128, dtype=f32)
    blob_f[:, 140:204] = c0[None, :]

    # bf16 blob (128, 448): cv | w2 | wo_exp | w1p (rows 0:64) | ident_bf16
    blob_b = np.zeros((128, 448), bf16)
    blob_b[:, 0:64] = Cv.astype(bf16)
    blob_b[:, 64:128] = w2.astype(bf16)
    blob_b[:, 128:192] = wo_exp.astype(bf16)
    blob_b[:64, 192:320] = w1p.astype(bf16)
    blob_b[:, 320:448] = np.eye(128, dtype=bf16)

    # bf16 row blob (1, 256): numbias_a/b | denbias_a/b
    blob_r = np.zeros((1, 256), bf16)
    blob_r[0, 0:64] = numbias[0].astype(bf16)
    blob_r[0, 64:128] = numbias[1].astype(bf16)
    blob_r[0, 128:192] = denbias[0].astype(bf16)
    blob_r[0, 192:256] = denbias[1].astype(bf16)

    return {"blob_f": blob_f, "blob_b": blob_b, "blob_r": blob_r}


def _build_bass():
    import concourse.bacc as bacc
    import concourse.mybir as mybir
    import concourse.tile as tile
    from concourse.masks import make_identity

    f32 = mybir.dt.float32
    bf16 = mybir.dt.bfloat16
    AF = mybir.ActivationFunctionType
    OP = mybir.AluOpType

    # Restrict Ln/Exp to the one table set containing both, so the
    # act-table-load pass cannot ping-pong between sets (each reload ~1.3us).
    # Set positions/ids are unchanged; we only narrow the choices.
    import concourse.hw_specs as hw_specs
    _orig_gat = hw_specs.get_activation_tables

    def _gat(arch):
        t = {k: set(v) for k, v in _orig_gat(arch).items()}
        for name, funcs in t.items():
            if name != "natural_log_exp_and_others":
                funcs.discard(AF.Exp)
                funcs.discard(AF.Ln)
        return t

    bacc.get_activation_tables = _gat

    nc = bacc.Bacc("TRN2", target_bir_lowering=False, debug=False,
                   num_devices=N_CORES)

    zs = nc.dram_tensor("zs", (NS, D_FEAT), f32, kind="ExternalInput")
    blob_f_d = nc.dram_tensor("blob_f", (128, 204), f32, kind="ExternalInput")
    blob_b_d = nc.dram_tensor("blob_b", (128, 448), bf16, kind="ExternalInput")
    blob_r_d = nc.dram_tensor("blob_r", (1, 256), bf16, kind="ExternalInput")
    yt = nc.dram_tensor("yt", (D_EMB, NS), f32, kind="ExternalOutput")

    with tile.TileContext(nc) as tc:
        with (
            tc.tile_pool(name="cp", bufs=1) as cp,
            tc.tile_pool(name="wk", bufs=1) as wkp,
            tc.tile_pool(name="sm", bufs=2) as sm,
            tc.tile_pool(name="hd", bufs=3) as hd,
            tc.tile_pool(name="ps", bufs=1, space="PSUM") as ps,
        ):
            # Z shard first, split per chunk, on the scalar queue (its
            # preamble ends earliest) so compute starts ASAP.
            za = wkp.tile([128, 4, D_FEAT], f32, tag="za")
            zre = zs.rearrange("(t p) f -> p t f", p=128)
            nc.sync.dma_start(out=za[:, 0:2, :], in_=zre[:, 0:2, :])
            nc.sync.dma_start(out=za[:, 2:4, :], in_=zre[:, 2:4, :])
            bf = cp.tile([128, 204], f32, tag="bf", name="bf")
            nc.sync.dma_start(out=bf, in_=blob_f_d[:])
            bb = cp.tile([128, 448], bf16, tag="bb", name="bb")
            nc.sync.dma_start(out=bb, in_=blob_b_d[:])
            br = cp.tile([1, 256], bf16, tag="br", name="br")
            nc.sync.dma_start(out=br, in_=blob_r_d[:])

            sqrtvf = bf[:, 0:1]
            gcol = bf[:, 1:5]
            dcol = bf[:, 5:9]
            b1p = bf[:, 9:10]
            c0b2 = bf[:64, 10:11]
            ident = bf[:, 12:140]
            c0b = bf[:, 140:204]
            cv = bb[:, 0:64]
            w2m = bb[:, 64:128]
            wo_m = bb[:, 128:192]
            w1p = bb[:64, 192:320]
            identb = bb[:, 320:448]

            ones_row = cp.tile([1, NS], bf16, tag="ones_row")
            nc.vector.memset(ones_row, 1.0)
            ones32 = cp.tile([128, 32], bf16, tag="ones32")
            nc.vector.memset(ones32, 1.0)
            eps_t = cp.tile([128, 1], f32, tag="eps_t")
            nc.vector.memset(eps_t, EPS)

            # ==== two node-chunks of 256, phase-interleaved so each engine
            # ping-pongs between chunks (software pipeline).
            NCH, CH, TPC = 2, NS // 2, 2
            C = range(NCH)
            st = {}

            # -- phase 1: LN(Z) stats
            for c in C:
                mvall = sm.tile([128, TPC, 2], f32, tag=f"mvall{c}", bufs=1,
                                name="mvall")
                st[c, "mv"] = mvall
                for t in range(TPC):
                    st6 = sm.tile([128, 6], f32, tag="st6", name="st6")
                    nc.vector.bn_stats(out=st6, in_=za[:, TPC * c + t, :])
                    nc.vector.bn_aggr(out=mvall[:, t, :], in_=st6)
            for c in C:
                lnv = sm.tile([128, TPC], f32, tag="lnv", name="lnv")
                nc.scalar.activation(out=lnv, in_=st[c, "mv"][:, :, 1],
                                     func=AF.Ln, bias=eps_t)
                rstd = sm.tile([128, TPC], f32, tag=f"rstd{c}", bufs=1,
                               name="rstd")
                nc.scalar.activation(out=rstd, in_=lnv, func=AF.Exp, scale=-0.5)
                st[c, "rstd"] = rstd

            # -- zn + transpose to layout B
            for c in C:
                znT_ps = ps.tile([128, CH], f32, tag=f"pA{c}", name="znT_ps")
                st[c, "znT"] = znT_ps
                for t in range(TPC):
                    zn = sm.tile([128, D_FEAT], f32, tag="zn", name="zn")
                    nc.vector.tensor_scalar(
                        out=zn, in0=za[:, TPC * c + t, :],
                        scalar1=st[c, "mv"][:, t, 0:1],
                        scalar2=st[c, "rstd"][:, t:t + 1],
                        op0=OP.subtract, op1=OP.mult)
                    nc.tensor.transpose(znT_ps[:, t * 128:(t + 1) * 128], zn,
                                        ident)

            # -- s = zn * rsqrt(zn^2*vf + eps)
            for c in C:
                zsq = wkp.tile([128, CH], f32, tag=f"zsq{c}", name="zsq")
                nc.scalar.activation(out=zsq, in_=st[c, "znT"], func=AF.Square,
                                     scale=sqrtvf, bias=0.0)
                lns = wkp.tile([128, CH], f32, tag=f"lns{c}", name="lns")
                nc.scalar.activation(out=lns, in_=zsq, func=AF.Ln, bias=eps_t)
                rr = wkp.tile([128, CH], f32, tag=f"rr{c}", name="rr")
                nc.scalar.activation(out=rr, in_=lns, func=AF.Exp, scale=-0.5)
                sT = wkp.tile([128, CH], bf16, tag=f"sT{c}", name="sT")
                nc.vector.tensor_mul(out=sT, in0=st[c, "znT"], in1=rr)
                st[c, "sT"] = sT

            # -- attention: head h -> pair pr=h//2 at base 32*(h%2)
            for c in C:
                st[c, "num"] = [
                    ps.tile([D_EMB, CH], f32, tag=f"pB{c}", name="num_psa"),
                    ps.tile([D_EMB, CH], f32, tag=f"pC{c}", name="num_psb")]
                st[c, "den"] = [
                    ps.tile([D_EMB, CH], f32, tag=f"pA{c}", name="den_psa"),
                    ps.tile([D_EMB, CH], f32, tag=f"pD{c}", name="den_psb")]
                for pr in (0, 1):
                    nc.tensor.matmul(st[c, "num"][pr][:, :],
                                     br[:, 64 * pr:64 * pr + 64],
                                     ones_row[:, :CH], start=True, stop=False)
                    nc.tensor.matmul(st[c, "den"][pr][:, :],
                                     br[:, 128 + 64 * pr:192 + 64 * pr],
                                     ones_row[:, :CH], start=True, stop=False)
            for h in range(4):
                pr, p = divmod(h, 2)
                for c in C:
                    eh = hd.tile([128, CH], bf16, tag="eh", name="eh")
                    nc.scalar.activation(out=eh, in_=st[c, "sT"], func=AF.Exp,
                                         scale=gcol[:, h:h + 1],
                                         bias=dcol[:, h:h + 1])
                    esh = hd.tile([128, CH], bf16, tag="esh", name="esh")
                    nc.vector.tensor_mul(out=esh, in0=eh, in1=st[c, "sT"])
                    nc.tensor.matmul(st[c, "num"][pr][32 * p:32 * p + DK, :],
                                     cv[:, h * DK:(h + 1) * DK], esh,
                                     start=False, stop=(p == 1))
                    nc.tensor.matmul(st[c, "den"][pr][32 * p:32 * p + 32, :],
                                     ones32, eh, start=False, stop=(p == 1))

            # -- oe = num/den (stacked pairs), x in both layouts
            for c in C:
                oe = wkp.tile([128, CH], bf16, tag=f"oe{c}", name="oe")
                st[c, "oe"] = oe
                for pr in (0, 1):
                    rcp = wkp.tile([D_EMB, CH], f32, tag=f"rcp{pr}", name="rcp")
                    nc.vector.reciprocal_approx_fast(out=rcp,
                                                     in_=st[c, "den"][pr])
                    nc.vector.tensor_mul(out=oe[64 * pr:64 * pr + 64, :],
                                         in0=st[c, "num"][pr], in1=rcp)
            for c in C:
                x_ps = ps.tile([D_EMB, CH], f32, tag=f"pD{c}", name="x_ps")
                st[c, "x"] = x_ps
                nc.tensor.matmul(x_ps, wo_m, st[c, "oe"], start=True,
                                 stop=False)
                xa_ps = ps.tile([128, TPC, D_EMB], f32, tag=f"pA{c}",
                                name="xa_ps")
                st[c, "xap"] = xa_ps
                for t in range(TPC):
                    nc.tensor.matmul(xa_ps[:, t, :],
                                     st[c, "oe"][:, t * 128:(t + 1) * 128],
                                     wo_m, start=True, stop=True)
            for c in C:
                xa = wkp.tile([128, TPC, D_EMB], f32, tag=f"xa{c}", name="xa")
                st[c, "xa"] = xa
                nc.vector.tensor_add(
                    out=xa, in0=st[c, "xap"],
                    in1=c0b.unsqueeze(1).to_broadcast((128, TPC, D_EMB)))

            # -- FFN layernorm stats
            for c in C:
                mvb = sm.tile([128, TPC, 2], f32, tag=f"mvb{c}", bufs=1,
                              name="mvb")
                st[c, "mvb"] = mvb
                for t in range(TPC):
                    st6b = sm.tile([128, 6], f32, tag="st6b", name="st6b")
                    nc.vector.bn_stats(out=st6b, in_=st[c, "xa"][:, t, :])
                    nc.vector.bn_aggr(out=mvb[:, t, :], in_=st6b)
            for c in C:
                lnvb = sm.tile([128, TPC], f32, tag="lnvb", name="lnvb")
                nc.scalar.activation(out=lnvb, in_=st[c, "mvb"][:, :, 1],
                                     func=AF.Ln, bias=eps_t)
                rstdb = sm.tile([128, TPC], f32, tag=f"rstdb{c}", bufs=1,
                                name="rstdb")
                nc.scalar.activation(out=rstdb, in_=lnvb, func=AF.Exp,
                                     scale=-0.5)
                st[c, "rstdb"] = rstdb

            # -- u-hat + transpose back to layout B
            for c in C:
                uT_ps = ps.tile([D_EMB, CH], bf16, tag=f"pB{c}", name="uT_ps")
                st[c, "uTp"] = uT_ps
                for t in range(TPC):
                    uh = sm.tile([128, D_EMB], bf16, tag="uh", name="uh")
                    nc.vector.tensor_scalar(
                        out=uh, in0=st[c, "xa"][:, t, :],
                        scalar1=st[c, "mvb"][:, t, 0:1],
                        scalar2=st[c, "rstdb"][:, t:t + 1],
                        op0=OP.subtract, op1=OP.mult)
                    nc.tensor.transpose(uT_ps[:, t * 128:(t + 1) * 128], uh,
                                        identb)
            for c in C:
                uT = wkp.tile([D_EMB, CH], bf16, tag=f"uT{c}", name="uT")
                st[c, "uT"] = uT
                nc.vector.tensor_copy(out=uT, in_=st[c, "uTp"])

            # -- FFN matmuls; w2 accumulates into x_ps; y = x_ps + (c0+b2)
            for c in C:
                h_ps = ps.tile([2 * D_EMB, CH], f32, tag=f"pC{c}", name="h_ps")
                st[c, "h"] = h_ps
                nc.tensor.matmul(h_ps, w1p, st[c, "uT"], start=True, stop=True)
            for c in C:
                hh = wkp.tile([2 * D_EMB, CH], bf16, tag=f"hh{c}", name="hh")
                nc.scalar.activation(out=hh, in_=st[c, "h"], func=AF.Gelu,
                                     bias=b1p)
                nc.tensor.matmul(st[c, "x"], w2m, hh, start=False, stop=True)
            for c in C:
                y_sb = wkp.tile([D_EMB, CH], f32, tag=f"y_sb{c}", name="y_sb")
                nc.vector.tensor_scalar_add(out=y_sb, in0=st[c, "x"],
                                            scalar1=c0b2)
                nc.sync.dma_start(out=yt[:, c * CH:(c + 1) * CH], in_=y_sb)

    nc.compile()
    return nc


def _get_nc():
    if "nc" not in _CACHE:
        _CACHE["nc"] = _build_bass()
    return _CACHE["nc"]


def kernel(Z, A_full, feat_emb, label_token, wq, bq, wk, bk, wv, bv, wo, bo,
           w1, b1, w2, b2, alpha, g1, be1, g2, be2, _trace=False,
           _trace_kwargs=None):
    from concourse.bass_utils import run_bass_kernel_spmd

    Z = np.ascontiguousarray(np.asarray(Z, dtype=np.float32))
    consts = _host_consts(
        np.asarray(A_full), np.asarray(feat_emb), np.asarray(label_token),
        np.asarray(wq), np.asarray(bq), np.asarray(wk), np.asarray(bk),
        np.asarray(wv), np.asarray(bv), np.asarray(wo), np.asarray(bo),
        np.asarray(w1), np.asarray(b1), np.asarray(w2), np.asarray(b2),
        np.asarray(alpha), np.asarray(g1), np.asarray(be1), np.asarray(g2),
        np.asarray(be2))
    consts = {k: np.ascontiguousarray(v) for k, v in consts.items()}

    nc = _get_nc()
    in_maps = []
    for c in range(N_CORES):
        m = dict(consts)
        m["zs"] = np.ascontiguousarray(Z[c * NS:(c + 1) * NS])
        in_maps.append(m)

    kw = {}
    if _trace:
        kw["trace"] = True
        if _trace_kwargs:
            kw.update(_trace_kwargs)
    res = run_bass_kernel_spmd(nc, in_maps, core_ids=list(range(N_CORES)), **kw)

    out = np.empty((N, D_EMB), np.float32)
    for c in range(N_CORES):
        out[c * NS:(c + 1) * NS] = res.results[c]["yt"].T
    if _trace:
        return out, res
    return out
